# revision 19
# baseline (speedup 1.0000x reference)
"""Multi-head self-attention Bass/Tile kernel for Trainium2, 8 NeuronCores.

Problem: B=4, S=2048, D=1024, H=16 heads (HD=64), fp32, causal mask,
no padding.  y = softmax((xWq+bq)(xWk+bk)^T / 8 + mask) (xWv+bv) Wo + bo

Sharding (4-way batch x 2-way head-group):
  core c -> batch b = c//2, head group g = c%2 (heads 8g..8g+7).
  Each core computes its 8 heads' attention output and a PARTIAL
  out-projection y_partial = attn_out @ Wout[rows of its heads] (+ bout
  on g==0 cores only).  Host sums the two partials per batch.

v2 design (all PE matmuls in bf16; fp32 PSUM accumulation):
  - x^T, W cast to bf16 on the host.  Q kept pair-packed [128,4,S]
    (head 2pr rows 0-63, head 2pr+1 rows 64-127).  K^T kept SBUF-resident
    zero-padded per parity slot [128,2,4,S] so score matmuls contract the
    full 128 partitions (keeps the PE HAM activity monitor warm).
  - Scores land in bf16 PSUM tiles [128, 2048] (2 banks, 4 chunk-slices)
    -> ONE exp per group on ACT (amortizes the ~352-cycle fixed cost).
  - Diagonal (causal edge) groups are column-packed: chunk m only computes
    q >= 128m (512/384/256/128 cols), exp over [128,1280], triangular
    0/1 bf16 multiply on the four 128-col edge blocks (GpSimd), so the
    always-masked region is never computed.
  - V is augmented with a ones-column: row 64 of the AV accumulation is
    the softmax denominator for free.  Per q-tile, denominators are
    gathered [8,512], reciprocal_approx_fast'd, cast to bf16, broadcast
    via a tiny PE matmul, and applied in-place to the unnormalized
    attn^T (bf16 SBUF).
  - Stage A (QKV proj, s-tile st), stage B (attention, q-tile qt=st-1),
    and stage C (out-proj) are interleaved per tile so the ACT exp stream
    overlaps the PE projection/out-proj matmuls.
"""

import sys

if "/opt/trn_rl_repo" not in sys.path:
    sys.path.insert(0, "/opt/trn_rl_repo")

import ml_dtypes
import numpy as np

import concourse.bass as bass
import concourse.mybir as mybir
import concourse.tile as tile
from concourse import bacc
from concourse.bass_utils import run_bass_kernel_spmd

f32 = mybir.dt.float32
BF16 = mybir.dt.bfloat16
AF = mybir.ActivationFunctionType
OP = mybir.AluOpType

B, S, D, H = 4, 2048, 1024, 16
HD = D // H            # 64
P = 128
DC = D // P            # 8 contraction chunks for the projections
NPAIR = 4              # head pairs per core (8 local heads)
NST = S // 512         # 4 s-tiles of 512
VW = HD + 8            # 72: V cols + one-hot cols -> per-head denominator rows
SCALE = 0.125          # 1/sqrt(64)

# diag packing across two fp32 PSUM tiles (bank = 512 fp32 cols):
# tile1 [128,1024]: m=0 at [0:512], m=1 at [512:896]
# tile2 [128,512]:  m=2 at [0:256], m=3 at [256:384]
DIAG1 = [(0, 0, 512, 0), (1, 512, 384, 128)]     # (m, packed_off, width, q_off)
DIAG2 = [(2, 0, 256, 256), (3, 256, 128, 384)]
DIAG1_W, DIAG2_W = 896, 384


def build_program():
    nc = bacc.Bacc("TRN2", target_bir_lowering=False, debug=False)

    xt_d = nc.dram_tensor("xT", [D, S], BF16, kind="ExternalInput")
    w_d = nc.dram_tensor("wqkv", [D, 3 * 512], BF16, kind="ExternalInput")
    b_d = nc.dram_tensor("bqkv", [3 * 512], f32, kind="ExternalInput")
    wo_d = nc.dram_tensor("wout", [512, D], BF16, kind="ExternalInput")
    bo_d = nc.dram_tensor("bout", [D], f32, kind="ExternalInput")
    tri_d = nc.dram_tensor("tri", [P, P], BF16, kind="ExternalInput")
    sel_d = nc.dram_tensor("sel", [8, 512], BF16, kind="ExternalInput")
    vm_d = nc.dram_tensor("vmask", [8, 8], BF16, kind="ExternalInput")
    y_d = nc.dram_tensor("y", [S, D], f32, kind="ExternalOutput")

    from contextlib import ExitStack

    with tile.TileContext(nc) as tc, ExitStack() as _lp:
        _lp.enter_context(
            nc.allow_low_precision(reason="bf16 matmuls/activations intended")
        )
        with tc.tile_pool(name="pers", bufs=1) as pers, \
             tc.tile_pool(name="consts", bufs=1) as consts, \
             tc.tile_pool(name="xtp", bufs=2) as xtp, \
             tc.tile_pool(name="ptp", bufs=3) as ptp, \
             tc.tile_pool(name="denp", bufs=2) as denp, \
             tc.tile_pool(name="ytp", bufs=2) as ytp, \
             tc.tile_pool(name="ps_acc", bufs=2, space="PSUM") as ps_acc, \
             tc.tile_pool(name="ps_sg", bufs=2, space="PSUM") as ps_sg, \
             tc.tile_pool(name="ps_av", bufs=2, space="PSUM") as ps_av:

            # ---- persistent activations ----
            # Q pair-packed: head 2pr rows 0-63, head 2pr+1 rows 64-127
            q_pair = pers.tile([P, NPAIR, S], BF16, tag="q")
            # K^T pair-packed like Q (head 2pr rows 0-63, 2pr+1 rows 64-127)
            kt_pair = pers.tile([P, NPAIR, S], BF16, tag="kt")
            # V natural layout per 128-row chunk + ones column
            v_all = pers.tile([P, S // P, 8, VW], BF16, tag="v")
            # normalized attn^T, same pair layout as Q
            attn_t = pers.tile([P, NPAIR, S], BF16, tag="attn")
            w_sb = pers.tile([P, DC, 3 * 512], BF16, tag="wqkv")
            wo_sb = pers.tile([P, 4, D], BF16, tag="wout")

            # ---- constants ----
            bq_sb = consts.tile([P, 12], f32, tag="bq")
            vb_sb = consts.tile([P, 512], f32, tag="vb")
            bo_sb = consts.tile([P, D], f32, tag="bo")
            tri_sb = consts.tile([P, P], BF16, tag="tri")
            sel_sb = consts.tile([8, 512], BF16, tag="sel")
            vm_sb = consts.tile([P, 8, 8], BF16, tag="vm")
            junk_sb = consts.tile([P, 8], BF16, tag="junk")

            # ---- init DMAs ----
            xt_r = xt_d.rearrange("(dc p) s -> p dc s", p=P)
            xts = {}

            def prefetch_xt(st):
                t = xtp.tile([P, DC, 512], BF16, tag="xt", name=f"xt{st}")
                nc.sync.dma_start(out=t[:], in_=xt_r[:, :, st * 512 : (st + 1) * 512])
                xts[st] = t

            # interleave W and x-tile-0 loads per contraction chunk so the
            # first Q/K chains stream right behind the DMAs
            xt0 = xtp.tile([P, DC, 512], BF16, tag="xt", name="xt0")
            for dc in range(DC):
                nc.sync.dma_start(
                    out=w_sb[:, dc, :], in_=w_d[dc * P : (dc + 1) * P, :]
                )
                nc.sync.dma_start(out=xt0[:, dc, :], in_=xt_r[:, dc, 0:512])
            xts[0] = xt0
            # constants + zero-fills on the GpSimd (SWDGE) queue
            nc.gpsimd.dma_start(out=bq_sb[:], in_=b_d.rearrange("(o p) -> p o", p=P))
            nc.gpsimd.dma_start(
                out=vb_sb[:], in_=b_d[None, 1024:1536].to_broadcast([P, 512])
            )
            nc.gpsimd.dma_start(out=tri_sb[:], in_=tri_d[:])
            nc.gpsimd.dma_start(out=sel_sb[:], in_=sel_d[:])
            nc.gpsimd.dma_start(
                out=vm_sb[:], in_=vm_d[None, :, :].to_broadcast([P, 8, 8])
            )
            nc.gpsimd.dma_start(
                out=bo_sb[:], in_=bo_d[None, :].to_broadcast([P, D])
            )
            for pc in range(4):
                nc.sync.dma_start(
                    out=wo_sb[:, pc, :], in_=wo_d[pc * P : (pc + 1) * P, :]
                )
            # one-hot denominator columns of V_aug: col 64+j = (j == h)
            for kc in range(S // P):
                nc.vector.tensor_copy(out=v_all[:, kc, :, HD:VW], in_=vm_sb[:])
            # pre-load the ACT exp table while DMAs run
            nc.scalar.activation(out=junk_sb[:], in_=vb_sb[:, 0:8], func=AF.Exp)

            # ================= stage helpers =================

            feed = []  # chain-granular PE work dripped between score groups
            pops = {"n": 0}

            def drip(n):
                for _ in range(n):
                    if feed:
                        feed.pop(0)()
                        pops["n"] += 1

            def drip_until(n):
                while pops["n"] < n and feed:
                    feed.pop(0)()
                    pops["n"] += 1

            def drip_paced(state):
                state["groups"] -= 1
                g = state["groups"]
                n = -(-len(feed) // g) if g > 0 else len(feed)
                drip(n)

            def queue_a(st):
                """Queue stage-A for s-tile st as whole-psum-chain thunks."""
                sl = slice(st * 512, (st + 1) * 512)

                def qk_chain(pr, which):
                    c0 = pr * P if which == 0 else 512 + pr * P

                    def f():
                        mm = ps_acc.tile([P, 512], f32, tag="acc", name="mm")
                        for dc in range(DC):
                            nc.tensor.matmul(
                                mm[:],
                                w_sb[:, dc, c0 : c0 + P],
                                xts[st][:, dc, :],
                                start=(dc == 0),
                                stop=(dc == DC - 1),
                            )
                        if which == 0:
                            nc.vector.tensor_scalar_add(
                                q_pair[:, pr, sl], mm[:], bq_sb[:, pr : pr + 1]
                            )
                        else:
                            nc.vector.tensor_scalar_add(
                                kt_pair[:, pr, sl], mm[:], bq_sb[:, 4 + pr : 5 + pr]
                            )
                    return f

                def v_chain(sb):
                    def f():
                        mm = ps_acc.tile([P, 512], f32, tag="acc", name="mmv")
                        for dc in range(DC):
                            nc.tensor.matmul(
                                mm[:],
                                xts[st][:, dc, sb * P : (sb + 1) * P],
                                w_sb[:, dc, 1024:1536],
                                start=(dc == 0),
                                stop=(dc == DC - 1),
                            )
                        nc.vector.tensor_tensor(
                            v_all[:, st * 4 + sb, :, 0:HD],
                            mm[:].rearrange("p (h d) -> p h d", h=8),
                            vb_sb[:].rearrange("p (h d) -> p h d", h=8),
                            OP.add,
                        )
                    return f

                feed.append(qk_chain(0, 0))
                feed.append(qk_chain(0, 1))
                for sb in range(4):
                    feed.append(v_chain(sb))
                for pr in range(1, NPAIR):
                    feed.append(qk_chain(pr, 0))
                    feed.append(qk_chain(pr, 1))

            def queue_c(qt):
                """Queue stage-C for q-tile qt: one chain per (q-chunk, half)."""
                for qc in range(4):
                    qq = qt * 512 + qc * P
                    yts = {}

                    def mk(nb, qq=qq, yts=yts):
                        def f():
                            if nb == 0:
                                yts["yt"] = ytp.tile([P, D], f32, tag="yt",
                                                     name="yt")
                            yp = ps_acc.tile([P, 512], f32, tag="acc", name="yp")
                            for pc in range(4):
                                nc.tensor.matmul(
                                    yp[:],
                                    attn_t[:, pc, qq : qq + P],
                                    wo_sb[:, pc, nb * 512 : (nb + 1) * 512],
                                    start=(pc == 0),
                                    stop=(pc == 3),
                                )
                            nc.vector.tensor_tensor(
                                yts["yt"][:, nb * 512 : (nb + 1) * 512],
                                yp[:],
                                bo_sb[:, nb * 512 : (nb + 1) * 512],
                                OP.add,
                            )
                            if nb == 1:
                                nc.sync.dma_start(
                                    out=y_d[qq : qq + P, :], in_=yts["yt"][:]
                                )
                        return f

                    feed.append(mk(0))
                    feed.append(mk(1))

            # pending AV groups            # pending AV groups            # pending AV groups            # pending AV groups            # pending AV groups: list of (av, h, chunks, pt, first, last, qt)
            pend = []

            def flush_pend(den_qt):
                while pend:
                    av, h, chunks, pt, first, last, qt = pend.pop(0)
                    n = len(chunks)
                    for i, (kc, po, w, qo) in enumerate(chunks):
                        nc.tensor.matmul(
                            av[:, qo : qo + w],
                            v_all[:, kc, h, :],
                            pt[:, po : po + w],
                            start=(first and i == 0),
                            stop=(last and i == n - 1),
                        )
                    if last:
                        par, pr = h % 2, h // 2
                        q0 = qt * 512
                        # den rows 64+h -> gather tile rows 0-7 (32-aligned)
                        if h == 0:
                            nc.vector.tensor_copy(
                                out=den_qt[0:8, :], in_=av[HD : HD + 8, :]
                            )
                        else:
                            nc.vector.tensor_tensor(
                                den_qt[0:8, :], den_qt[0:8, :],
                                av[HD : HD + 8, :], OP.add,
                            )
                        # unnormalized attn^T -> SBUF bf16
                        nc.vector.tensor_copy(
                            out=attn_t[HD * par : HD * par + HD, pr, q0 : q0 + 512],
                            in_=av[0:HD, :],
                        )

            def b_pair(qt, pr, den_qt, pace):
                """Both heads of pair pr: row-tiled concurrent score matmuls
                (head 2pr on array rows 0-63, head 2pr+1 on rows 64-127)."""
                q0 = qt * 512
                avs = [ps_av.tile([VW, 512], f32, tag="av", name=f"av{par}")
                       for par in range(2)]

                def paired_scores(pool, tag, width, chunks):
                    sgs = [pool.tile([P, width], f32, tag=tag, name=f"sg{par}")
                           for par in range(2)]
                    for kc, po, w, qo in chunks:
                        for par in range(2):
                            lo = HD * par
                            nc.tensor.matmul(
                                sgs[par][:, po : po + w],
                                kt_pair[lo : lo + HD, pr, kc * P : (kc + 1) * P],
                                q_pair[lo : lo + HD, pr, q0 + qo : q0 + 512],
                                start=True,
                                stop=True,
                            )
                    return sgs

                # full groups of 2 chunks
                for g in range(2 * qt):
                    chunks = [(2 * g + c, 512 * c, 512, 0) for c in range(2)]
                    sgs = paired_scores(ps_sg, "sg", 1024, chunks)
                    for par in range(2):
                        pt = ptp.tile([P, 1024], BF16, tag="pt", name="pt")
                        nc.scalar.activation(
                            out=pt[:], in_=sgs[par][:], func=AF.Exp, scale=SCALE
                        )
                        flush_pend(den_qt)
                        pend.append(
                            (avs[par], 2 * pr + par, list(chunks),
                             pt, g == 0, False, qt)
                        )
                    drip_paced(pace)
                # diag chunks, column-packed into two tiles per head
                for di, (spec, wtot, last) in enumerate(
                    [(DIAG1, DIAG1_W, False), (DIAG2, DIAG2_W, True)]
                ):
                    chunks = [(4 * qt + m, po, w, qo) for m, po, w, qo in spec]
                    sgs = paired_scores(ps_sg, "sg", 1024, chunks)
                    for par in range(2):
                        ptd = ptp.tile([P, wtot], BF16, tag="pt", name="ptd")
                        nc.scalar.activation(
                            out=ptd[:], in_=sgs[par][:, 0:wtot],
                            func=AF.Exp, scale=SCALE,
                        )
                        # zero the future (upper-triangle) edge of each chunk
                        for kc, po, w, qo in chunks:
                            nc.gpsimd.tensor_tensor(
                                ptd[:, po : po + P], ptd[:, po : po + P],
                                tri_sb[:], OP.mult,
                            )
                        flush_pend(den_qt)
                        pend.append(
                            (avs[par], 2 * pr + par, chunks, ptd,
                             qt == 0 and di == 0, last, qt)
                        )
                    drip_paced(pace)

            def b_norm(qt, den_qt):
                """B2+B3: reciprocal + pair broadcast + in-place normalize."""
                den_bf = denp.tile([8, 512], BF16, tag="den_bf", name="den_bf")
                nc.vector.reciprocal_approx_fast(out=den_qt[:], in_=den_qt[:])
                nc.vector.tensor_copy(out=den_bf[:], in_=den_qt[:])
                q0 = qt * 512
                for pr in range(NPAIR):
                    # rb rows 0-63 <- recip den head 2pr, rows 64-127 <- 2pr+1
                    rb = ps_acc.tile([P, 512], f32, tag="acc", name="rb")
                    nc.tensor.matmul(
                        rb[:],
                        sel_sb[:, pr * P : (pr + 1) * P],
                        den_bf[:],
                        start=True,
                        stop=True,
                    )
                    nc.vector.tensor_tensor(
                        attn_t[:, pr, q0 : q0 + 512],
                        attn_t[:, pr, q0 : q0 + 512],
                        rb[:],
                        OP.mult,
                    )

            def c_tile(qt):
                for qc in range(4):
                    qq = qt * 512 + qc * P
                    yt = ytp.tile([P, D], f32, tag="yt", name="yt")
                    for nb in range(2):
                        yp = ps_acc.tile([P, 512], f32, tag="acc", name="yp")
                        for pc in range(4):
                            nc.tensor.matmul(
                                yp[:],
                                attn_t[:, pc, qq : qq + P],
                                wo_sb[:, pc, nb * 512 : (nb + 1) * 512],
                                start=(pc == 0),
                                stop=(pc == 3),
                            )
                        nc.vector.tensor_tensor(
                            yt[:, nb * 512 : (nb + 1) * 512],
                            yp[:],
                            bo_sb[:, nb * 512 : (nb + 1) * 512],
                            OP.add,
                        )
                    nc.sync.dma_start(out=y_d[qq : qq + P, :], in_=yt[:])

            # ================= interleaved schedule =================
            # Round qt runs B(qt); stage A(qt+1) / out-proj chains drip
            # between score groups.  Out-projections go to ACT-bound round 3.
            queue_a(0)

            den_tiles = {}
            for qt in range(NST):
                if qt + 1 < NST:
                    prefetch_xt(qt + 1)
                    queue_a(qt + 1)
                if qt == NST - 1:
                    queue_c(0)
                    queue_c(1)
                den_qt = denp.tile([8, 512], f32, tag="den", name=f"den{qt}")
                den_tiles[qt] = den_qt
                pace = {"groups": 4 * (2 * qt + 2)}
                for pr in range(NPAIR):
                    if qt == 0:
                        drip_until(6 + 2 * pr)
                    b_pair(qt, pr, den_qt, pace)
                    if pr == 0 and qt >= 1:
                        flush_pend(den_qt)
                        b_norm(qt - 1, den_tiles[qt - 1])
                        if qt == NST - 1:
                            queue_c(qt - 1)
                flush_pend(den_qt)
                drip(len(feed))
                xts.pop(qt, None)
            b_norm(NST - 1, den_tiles[NST - 1])
            queue_c(NST - 1)
            drip(len(feed))

    nc.finalize()
    return nc


_NC = None


def _get_nc():
    global _NC
    if _NC is None:
        _NC = build_program()
    return _NC


def _shard_inputs(x, causal_mask, padding_mask, W_qkv, b_qkv, W_out, b_out):
    bf16 = ml_dtypes.bfloat16
    x = np.ascontiguousarray(np.asarray(x, dtype=np.float32))
    W_qkv = np.asarray(W_qkv, dtype=np.float32)
    b_qkv = np.asarray(b_qkv, dtype=np.float32)
    W_out = np.asarray(W_out, dtype=np.float32)
    b_out = np.asarray(b_out, dtype=np.float32)
    padding_mask = np.asarray(padding_mask)

    assert not padding_mask.any(), "kernel assumes no padding"

    # multiplicative 0/1 triangle: keep score^T[k, q'] iff q' >= k
    tri = np.triu(np.ones((P, P), dtype=np.float32)).astype(bf16)
    # sel[:, pr*128:(pr+1)*128]: cols 0-63 pick head 2pr, 64-127 pick 2pr+1
    sel = np.zeros((8, 512), dtype=np.float32)
    for pr in range(4):
        sel[2 * pr, pr * P : pr * P + HD] = 1.0
        sel[2 * pr + 1, pr * P + HD : (pr + 1) * P] = 1.0
    sel = sel.astype(bf16)

    in_maps = []
    for c in range(8):
        b, g = c // 2, c % 2
        cols = slice(g * 512, (g + 1) * 512)
        w_slice = np.concatenate(
            [W_qkv[:, cols], W_qkv[:, 1024:2048][:, cols], W_qkv[:, 2048:3072][:, cols]],
            axis=1,
        )
        b_slice = np.concatenate(
            [b_qkv[cols], b_qkv[1024:2048][cols], b_qkv[2048:3072][cols]]
        )
        in_maps.append(
            {
                "xT": np.ascontiguousarray(x[b].T).astype(bf16),
                "wqkv": np.ascontiguousarray(w_slice).astype(bf16),
                "bqkv": np.ascontiguousarray(b_slice),
                "wout": np.ascontiguousarray(W_out[g * 512 : (g + 1) * 512, :]).astype(bf16),
                "bout": b_out if g == 0 else np.zeros_like(b_out),
                "tri": tri,
                "sel": sel,
                "vmask": np.eye(8, dtype=np.float32).astype(bf16),
            }
        )
    return in_maps


def _run(in_maps, **kwargs):
    nc = _get_nc()
    return run_bass_kernel_spmd(nc, in_maps, core_ids=list(range(8)), **kwargs)


def kernel(**inputs):
    in_maps = _shard_inputs(**inputs)
    res = _run(in_maps)
    out = np.empty((B, S, D), dtype=np.float32)
    for b in range(B):
        out[b] = res.results[2 * b]["y"] + res.results[2 * b + 1]["y"]
    return out


def kernel_traced(**inputs):
    """Like kernel() but with NTFF tracing; returns (out, BassKernelResults)."""
    in_maps = _shard_inputs(**inputs)
    res = _run(in_maps, trace=True)
    out = np.empty((B, S, D), dtype=np.float32)
    for b in range(B):
        out[b] = res.results[2 * b]["y"] + res.results[2 * b + 1]["y"]
    return out, res


# revision 20
# speedup vs baseline: 1.0179x; 1.0179x over previous
"""Multi-head self-attention Bass/Tile kernel for Trainium2, 8 NeuronCores.

Problem: B=4, S=2048, D=1024, H=16 heads (HD=64), fp32, causal mask,
no padding.  y = softmax((xWq+bq)(xWk+bk)^T / 8 + mask) (xWv+bv) Wo + bo

Sharding (4-way batch x 2-way head-group):
  core c -> batch b = c//2, head group g = c%2 (heads 8g..8g+7).
  Each core computes its 8 heads' attention output and a PARTIAL
  out-projection y_partial = attn_out @ Wout[rows of its heads] (+ bout
  on g==0 cores only).  Host sums the two partials per batch.

v2 design (all PE matmuls in bf16; fp32 PSUM accumulation):
  - x^T, W cast to bf16 on the host.  Q kept pair-packed [128,4,S]
    (head 2pr rows 0-63, head 2pr+1 rows 64-127).  K^T kept SBUF-resident
    zero-padded per parity slot [128,2,4,S] so score matmuls contract the
    full 128 partitions (keeps the PE HAM activity monitor warm).
  - Scores land in bf16 PSUM tiles [128, 2048] (2 banks, 4 chunk-slices)
    -> ONE exp per group on ACT (amortizes the ~352-cycle fixed cost).
  - Diagonal (causal edge) groups are column-packed: chunk m only computes
    q >= 128m (512/384/256/128 cols), exp over [128,1280], triangular
    0/1 bf16 multiply on the four 128-col edge blocks (GpSimd), so the
    always-masked region is never computed.
  - V is augmented with a ones-column: row 64 of the AV accumulation is
    the softmax denominator for free.  Per q-tile, denominators are
    gathered [8,512], reciprocal_approx_fast'd, cast to bf16, broadcast
    via a tiny PE matmul, and applied in-place to the unnormalized
    attn^T (bf16 SBUF).
  - Stage A (QKV proj, s-tile st), stage B (attention, q-tile qt=st-1),
    and stage C (out-proj) are interleaved per tile so the ACT exp stream
    overlaps the PE projection/out-proj matmuls.
"""

import sys

if "/opt/trn_rl_repo" not in sys.path:
    sys.path.insert(0, "/opt/trn_rl_repo")

import ml_dtypes
import numpy as np

import concourse.bass as bass
import concourse.mybir as mybir
import concourse.tile as tile
from concourse import bacc
from concourse.bass_utils import run_bass_kernel_spmd

f32 = mybir.dt.float32
BF16 = mybir.dt.bfloat16
AF = mybir.ActivationFunctionType
OP = mybir.AluOpType

B, S, D, H = 4, 2048, 1024, 16
HD = D // H            # 64
P = 128
DC = D // P            # 8 contraction chunks for the projections
NPAIR = 4              # head pairs per core (8 local heads)
NST = S // 512         # 4 s-tiles of 512
VW = HD + 8            # 72: V cols + one-hot cols -> per-head denominator rows
SCALE = 0.125          # 1/sqrt(64)

# diag packing across two fp32 PSUM tiles (bank = 512 fp32 cols):
# tile1 [128,1024]: m=0 at [0:512], m=1 at [512:896]
# tile2 [128,512]:  m=2 at [0:256], m=3 at [256:384]
DIAG1 = [(0, 0, 512, 0), (1, 512, 384, 128)]     # (m, packed_off, width, q_off)
DIAG2 = [(2, 0, 256, 256), (3, 256, 128, 384)]
DIAG1_W, DIAG2_W = 896, 384


def build_program():
    nc = bacc.Bacc("TRN2", target_bir_lowering=False, debug=False)

    xt_d = nc.dram_tensor("xT", [D, S], BF16, kind="ExternalInput")
    w_d = nc.dram_tensor("wqkv", [D, 3 * 512], BF16, kind="ExternalInput")
    b_d = nc.dram_tensor("bqkv", [3 * 512], f32, kind="ExternalInput")
    wo_d = nc.dram_tensor("wout", [512, D], BF16, kind="ExternalInput")
    bo_d = nc.dram_tensor("bout", [D], f32, kind="ExternalInput")
    tri_d = nc.dram_tensor("tri", [P, P], BF16, kind="ExternalInput")
    sel_d = nc.dram_tensor("sel", [8, 512], BF16, kind="ExternalInput")
    vm_d = nc.dram_tensor("vmask", [8, 8], BF16, kind="ExternalInput")
    y_d = nc.dram_tensor("y", [S, D], f32, kind="ExternalOutput")

    from contextlib import ExitStack

    with tile.TileContext(nc) as tc, ExitStack() as _lp:
        _lp.enter_context(
            nc.allow_low_precision(reason="bf16 matmuls/activations intended")
        )
        with tc.tile_pool(name="pers", bufs=1) as pers, \
             tc.tile_pool(name="consts", bufs=1) as consts, \
             tc.tile_pool(name="xtp", bufs=2) as xtp, \
             tc.tile_pool(name="ptp", bufs=3) as ptp, \
             tc.tile_pool(name="denp", bufs=2) as denp, \
             tc.tile_pool(name="ytp", bufs=2) as ytp, \
             tc.tile_pool(name="ps_acc", bufs=2, space="PSUM") as ps_acc, \
             tc.tile_pool(name="ps_sg", bufs=2, space="PSUM") as ps_sg, \
             tc.tile_pool(name="ps_av", bufs=2, space="PSUM") as ps_av:

            # ---- persistent activations ----
            # Q pair-packed: head 2pr rows 0-63, head 2pr+1 rows 64-127
            q_pair = pers.tile([P, NPAIR, S], BF16, tag="q")
            # K^T pair-packed like Q (head 2pr rows 0-63, 2pr+1 rows 64-127)
            kt_pair = pers.tile([P, NPAIR, S], BF16, tag="kt")
            # V natural layout per 128-row chunk + ones column
            v_all = pers.tile([P, S // P, 8, VW], BF16, tag="v")
            # normalized attn^T, same pair layout as Q
            attn_t = pers.tile([P, NPAIR, S], BF16, tag="attn")
            w_sb = pers.tile([P, DC, 3 * 512], BF16, tag="wqkv")
            wo_sb = pers.tile([P, 4, D], BF16, tag="wout")

            # ---- constants ----
            bq_sb = consts.tile([P, 12], f32, tag="bq")
            vb_sb = consts.tile([P, 512], f32, tag="vb")
            bo_sb = consts.tile([P, D], f32, tag="bo")
            tri_sb = consts.tile([P, P], BF16, tag="tri")
            sel_sb = consts.tile([8, 512], BF16, tag="sel")
            vm_sb = consts.tile([P, 8, 8], BF16, tag="vm")
            junk_sb = consts.tile([P, 8], BF16, tag="junk")

            # ---- init DMAs ----
            xt_r = xt_d.rearrange("(dc p) s -> p dc s", p=P)
            xts = {}

            def prefetch_xt(st):
                t = xtp.tile([P, DC, 512], BF16, tag="xt", name=f"xt{st}")
                nc.sync.dma_start(out=t[:], in_=xt_r[:, :, st * 512 : (st + 1) * 512])
                xts[st] = t

            # interleave W and x-tile-0 loads per contraction chunk so the
            # first Q/K chains stream right behind the DMAs
            xt0 = xtp.tile([P, DC, 512], BF16, tag="xt", name="xt0")
            for dc in range(DC):
                nc.sync.dma_start(
                    out=w_sb[:, dc, :], in_=w_d[dc * P : (dc + 1) * P, :]
                )
                nc.sync.dma_start(out=xt0[:, dc, :], in_=xt_r[:, dc, 0:512])
            xts[0] = xt0
            # constants + zero-fills on the GpSimd (SWDGE) queue
            nc.gpsimd.dma_start(out=bq_sb[:], in_=b_d.rearrange("(o p) -> p o", p=P))
            nc.gpsimd.dma_start(
                out=vb_sb[:], in_=b_d[None, 1024:1536].to_broadcast([P, 512])
            )
            nc.gpsimd.dma_start(out=tri_sb[:], in_=tri_d[:])
            nc.gpsimd.dma_start(out=sel_sb[:], in_=sel_d[:])
            nc.gpsimd.dma_start(
                out=vm_sb[:], in_=vm_d[None, :, :].to_broadcast([P, 8, 8])
            )
            nc.gpsimd.dma_start(
                out=bo_sb[:], in_=bo_d[None, :].to_broadcast([P, D])
            )
            for pc in range(4):
                nc.sync.dma_start(
                    out=wo_sb[:, pc, :], in_=wo_d[pc * P : (pc + 1) * P, :]
                )
            # one-hot denominator columns of V_aug: col 64+j = (j == h)
            for kc in range(S // P):
                nc.vector.tensor_copy(out=v_all[:, kc, :, HD:VW], in_=vm_sb[:])
            # pre-load the ACT exp table while DMAs run
            nc.scalar.activation(out=junk_sb[:], in_=vb_sb[:, 0:8], func=AF.Exp)

            # ================= stage helpers =================

            feed = []  # chain-granular PE work dripped between score groups
            pops = {"n": 0}

            def drip(n):
                for _ in range(n):
                    if feed:
                        feed.pop(0)()
                        pops["n"] += 1

            def drip_until(n):
                while pops["n"] < n and feed:
                    feed.pop(0)()
                    pops["n"] += 1

            def drip_paced(state):
                state["groups"] -= 1
                g = state["groups"]
                avail = max(0, len(feed) - state.get("reserve", 0))
                n = -(-avail // g) if g > 0 else avail
                drip(n)

            def queue_a(st):
                """Queue stage-A for s-tile st as whole-psum-chain thunks."""
                sl = slice(st * 512, (st + 1) * 512)

                def qk_chain(pr, which):
                    c0 = pr * P if which == 0 else 512 + pr * P

                    def f():
                        mm = ps_acc.tile([P, 512], f32, tag="acc", name="mm")
                        for dc in range(DC):
                            nc.tensor.matmul(
                                mm[:],
                                w_sb[:, dc, c0 : c0 + P],
                                xts[st][:, dc, :],
                                start=(dc == 0),
                                stop=(dc == DC - 1),
                            )
                        if which == 0:
                            nc.vector.tensor_scalar_add(
                                q_pair[:, pr, sl], mm[:], bq_sb[:, pr : pr + 1]
                            )
                        else:
                            nc.vector.tensor_scalar_add(
                                kt_pair[:, pr, sl], mm[:], bq_sb[:, 4 + pr : 5 + pr]
                            )
                    return f

                def v_chain(sb):
                    def f():
                        mm = ps_acc.tile([P, 512], f32, tag="acc", name="mmv")
                        for dc in range(DC):
                            nc.tensor.matmul(
                                mm[:],
                                xts[st][:, dc, sb * P : (sb + 1) * P],
                                w_sb[:, dc, 1024:1536],
                                start=(dc == 0),
                                stop=(dc == DC - 1),
                            )
                        nc.vector.tensor_tensor(
                            v_all[:, st * 4 + sb, :, 0:HD],
                            mm[:].rearrange("p (h d) -> p h d", h=8),
                            vb_sb[:].rearrange("p (h d) -> p h d", h=8),
                            OP.add,
                        )
                    return f

                feed.append(qk_chain(0, 0))
                feed.append(qk_chain(0, 1))
                for sb in range(4):
                    feed.append(v_chain(sb))
                for pr in range(1, NPAIR):
                    feed.append(qk_chain(pr, 0))
                    feed.append(qk_chain(pr, 1))

            def queue_c(qt):
                """Queue stage-C for q-tile qt: one chain per (q-chunk, half)."""
                for qc in range(4):
                    qq = qt * 512 + qc * P
                    yts = {}

                    def mk(nb, qq=qq, yts=yts):
                        def f():
                            if nb == 0:
                                yts["yt"] = ytp.tile([P, D], f32, tag="yt",
                                                     name="yt")
                            yp = ps_acc.tile([P, 512], f32, tag="acc", name="yp")
                            for pc in range(4):
                                nc.tensor.matmul(
                                    yp[:],
                                    attn_t[:, pc, qq : qq + P],
                                    wo_sb[:, pc, nb * 512 : (nb + 1) * 512],
                                    start=(pc == 0),
                                    stop=(pc == 3),
                                )
                            nc.vector.tensor_tensor(
                                yts["yt"][:, nb * 512 : (nb + 1) * 512],
                                yp[:],
                                bo_sb[:, nb * 512 : (nb + 1) * 512],
                                OP.add,
                            )
                            if nb == 1:
                                nc.sync.dma_start(
                                    out=y_d[qq : qq + P, :], in_=yts["yt"][:]
                                )
                        return f

                    feed.append(mk(0))
                    feed.append(mk(1))

            # pending AV groups            # pending AV groups            # pending AV groups            # pending AV groups            # pending AV groups: list of (av, h, chunks, pt, first, last, qt)
            pend = []

            def flush_pend(den_qt):
                while pend:
                    av, h, chunks, pt, first, last, qt = pend.pop(0)
                    n = len(chunks)
                    for i, (kc, po, w, qo) in enumerate(chunks):
                        nc.tensor.matmul(
                            av[:, qo : qo + w],
                            v_all[:, kc, h, :],
                            pt[:, po : po + w],
                            start=(first and i == 0),
                            stop=(last and i == n - 1),
                        )
                    if last:
                        par, pr = h % 2, h // 2
                        q0 = qt * 512
                        # den rows 64+h -> gather tile rows 0-7 (32-aligned)
                        if h == 0:
                            nc.vector.tensor_copy(
                                out=den_qt[0:8, :], in_=av[HD : HD + 8, :]
                            )
                        else:
                            nc.vector.tensor_tensor(
                                den_qt[0:8, :], den_qt[0:8, :],
                                av[HD : HD + 8, :], OP.add,
                            )
                        # unnormalized attn^T -> SBUF bf16
                        nc.vector.tensor_copy(
                            out=attn_t[HD * par : HD * par + HD, pr, q0 : q0 + 512],
                            in_=av[0:HD, :],
                        )

            def b_pair(qt, pr, den_qt, pace):
                """Both heads of pair pr: row-tiled concurrent score matmuls
                (head 2pr on array rows 0-63, head 2pr+1 on rows 64-127)."""
                q0 = qt * 512
                avs = [ps_av.tile([VW, 512], f32, tag="av", name=f"av{par}")
                       for par in range(2)]

                def paired_scores(pool, tag, width, chunks):
                    sgs = [pool.tile([P, width], f32, tag=tag, name=f"sg{par}")
                           for par in range(2)]
                    for kc, po, w, qo in chunks:
                        for par in range(2):
                            lo = HD * par
                            nc.tensor.matmul(
                                sgs[par][:, po : po + w],
                                kt_pair[lo : lo + HD, pr, kc * P : (kc + 1) * P],
                                q_pair[lo : lo + HD, pr, q0 + qo : q0 + 512],
                                start=True,
                                stop=True,
                            )
                    return sgs

                # full groups of 2 chunks
                for g in range(2 * qt):
                    chunks = [(2 * g + c, 512 * c, 512, 0) for c in range(2)]
                    sgs = paired_scores(ps_sg, "sg", 1024, chunks)
                    for par in range(2):
                        pt = ptp.tile([P, 1024], BF16, tag="pt", name="pt")
                        nc.scalar.activation(
                            out=pt[:], in_=sgs[par][:], func=AF.Exp, scale=SCALE
                        )
                        flush_pend(den_qt)
                        pend.append(
                            (avs[par], 2 * pr + par, list(chunks),
                             pt, g == 0, False, qt)
                        )
                    drip_paced(pace)
                # diag chunks, column-packed into two tiles per head
                for di, (spec, wtot, last) in enumerate(
                    [(DIAG1, DIAG1_W, False), (DIAG2, DIAG2_W, True)]
                ):
                    chunks = [(4 * qt + m, po, w, qo) for m, po, w, qo in spec]
                    sgs = paired_scores(ps_sg, "sg", 1024, chunks)
                    for par in range(2):
                        ptd = ptp.tile([P, wtot], BF16, tag="pt", name="ptd")
                        nc.scalar.activation(
                            out=ptd[:], in_=sgs[par][:, 0:wtot],
                            func=AF.Exp, scale=SCALE,
                        )
                        # zero the future (upper-triangle) edge of each chunk
                        for kc, po, w, qo in chunks:
                            nc.gpsimd.tensor_tensor(
                                ptd[:, po : po + P], ptd[:, po : po + P],
                                tri_sb[:], OP.mult,
                            )
                        flush_pend(den_qt)
                        pend.append(
                            (avs[par], 2 * pr + par, chunks, ptd,
                             qt == 0 and di == 0, last, qt)
                        )
                    drip_paced(pace)

            def b_norm(qt, den_qt):
                """B2+B3: reciprocal + pair broadcast + in-place normalize."""
                den_bf = denp.tile([8, 512], BF16, tag="den_bf", name="den_bf")
                nc.vector.reciprocal_approx_fast(out=den_qt[:], in_=den_qt[:])
                nc.vector.tensor_copy(out=den_bf[:], in_=den_qt[:])
                q0 = qt * 512
                for pr in range(NPAIR):
                    # rb rows 0-63 <- recip den head 2pr, rows 64-127 <- 2pr+1
                    rb = ps_acc.tile([P, 512], f32, tag="acc", name="rb")
                    nc.tensor.matmul(
                        rb[:],
                        sel_sb[:, pr * P : (pr + 1) * P],
                        den_bf[:],
                        start=True,
                        stop=True,
                    )
                    nc.vector.tensor_tensor(
                        attn_t[:, pr, q0 : q0 + 512],
                        attn_t[:, pr, q0 : q0 + 512],
                        rb[:],
                        OP.mult,
                    )

            def c_tile(qt):
                for qc in range(4):
                    qq = qt * 512 + qc * P
                    yt = ytp.tile([P, D], f32, tag="yt", name="yt")
                    for nb in range(2):
                        yp = ps_acc.tile([P, 512], f32, tag="acc", name="yp")
                        for pc in range(4):
                            nc.tensor.matmul(
                                yp[:],
                                attn_t[:, pc, qq : qq + P],
                                wo_sb[:, pc, nb * 512 : (nb + 1) * 512],
                                start=(pc == 0),
                                stop=(pc == 3),
                            )
                        nc.vector.tensor_tensor(
                            yt[:, nb * 512 : (nb + 1) * 512],
                            yp[:],
                            bo_sb[:, nb * 512 : (nb + 1) * 512],
                            OP.add,
                        )
                    nc.sync.dma_start(out=y_d[qq : qq + P, :], in_=yt[:])

            # ================= interleaved schedule =================
            # Round qt runs B(qt); stage A(qt+1) / out-proj chains drip
            # between score groups.  Out-projections go to ACT-bound round 3.
            queue_a(0)

            den_tiles = {}
            for qt in range(NST):
                if qt + 1 < NST:
                    prefetch_xt(qt + 1)
                    queue_a(qt + 1)
                if qt == NST - 1:
                    queue_c(0)
                    queue_c(1)
                den_qt = denp.tile([8, 512], f32, tag="den", name=f"den{qt}")
                den_tiles[qt] = den_qt
                pace = {"groups": 4 * (2 * qt + 2), "reserve": 2}
                for pr in range(NPAIR):
                    if qt == 0:
                        drip_until(6 + 2 * pr)
                    b_pair(qt, pr, den_qt, pace)
                    if pr == 0 and qt >= 1:
                        flush_pend(den_qt)
                        b_norm(qt - 1, den_tiles[qt - 1])
                        if qt == NST - 1:
                            queue_c(qt - 1)
                drip(len(feed))
                flush_pend(den_qt)
                xts.pop(qt, None)
            b_norm(NST - 1, den_tiles[NST - 1])
            queue_c(NST - 1)
            drip(len(feed))

    nc.finalize()
    return nc


_NC = None


def _get_nc():
    global _NC
    if _NC is None:
        _NC = build_program()
    return _NC


def _shard_inputs(x, causal_mask, padding_mask, W_qkv, b_qkv, W_out, b_out):
    bf16 = ml_dtypes.bfloat16
    x = np.ascontiguousarray(np.asarray(x, dtype=np.float32))
    W_qkv = np.asarray(W_qkv, dtype=np.float32)
    b_qkv = np.asarray(b_qkv, dtype=np.float32)
    W_out = np.asarray(W_out, dtype=np.float32)
    b_out = np.asarray(b_out, dtype=np.float32)
    padding_mask = np.asarray(padding_mask)

    assert not padding_mask.any(), "kernel assumes no padding"

    # multiplicative 0/1 triangle: keep score^T[k, q'] iff q' >= k
    tri = np.triu(np.ones((P, P), dtype=np.float32)).astype(bf16)
    # sel[:, pr*128:(pr+1)*128]: cols 0-63 pick head 2pr, 64-127 pick 2pr+1
    sel = np.zeros((8, 512), dtype=np.float32)
    for pr in range(4):
        sel[2 * pr, pr * P : pr * P + HD] = 1.0
        sel[2 * pr + 1, pr * P + HD : (pr + 1) * P] = 1.0
    sel = sel.astype(bf16)

    in_maps = []
    for c in range(8):
        b, g = c // 2, c % 2
        cols = slice(g * 512, (g + 1) * 512)
        w_slice = np.concatenate(
            [W_qkv[:, cols], W_qkv[:, 1024:2048][:, cols], W_qkv[:, 2048:3072][:, cols]],
            axis=1,
        )
        b_slice = np.concatenate(
            [b_qkv[cols], b_qkv[1024:2048][cols], b_qkv[2048:3072][cols]]
        )
        in_maps.append(
            {
                "xT": np.ascontiguousarray(x[b].T).astype(bf16),
                "wqkv": np.ascontiguousarray(w_slice).astype(bf16),
                "bqkv": np.ascontiguousarray(b_slice),
                "wout": np.ascontiguousarray(W_out[g * 512 : (g + 1) * 512, :]).astype(bf16),
                "bout": b_out if g == 0 else np.zeros_like(b_out),
                "tri": tri,
                "sel": sel,
                "vmask": np.eye(8, dtype=np.float32).astype(bf16),
            }
        )
    return in_maps


def _run(in_maps, **kwargs):
    nc = _get_nc()
    return run_bass_kernel_spmd(nc, in_maps, core_ids=list(range(8)), **kwargs)


def kernel(**inputs):
    in_maps = _shard_inputs(**inputs)
    res = _run(in_maps)
    out = np.empty((B, S, D), dtype=np.float32)
    for b in range(B):
        out[b] = res.results[2 * b]["y"] + res.results[2 * b + 1]["y"]
    return out


def kernel_traced(**inputs):
    """Like kernel() but with NTFF tracing; returns (out, BassKernelResults)."""
    in_maps = _shard_inputs(**inputs)
    res = _run(in_maps, trace=True)
    out = np.empty((B, S, D), dtype=np.float32)
    for b in range(B):
        out[b] = res.results[2 * b]["y"] + res.results[2 * b + 1]["y"]
    return out, res
